# revision 15
# baseline (speedup 1.0000x reference)
"""Paged causal GQA attention on 8 TRN2 NeuronCores.

Problem (hardcoded): B=8 seqs x S=1024 tokens, H=32 q-heads, KVH=8 kv-heads
(GQA group 4), D=128, f32 in/out, paged KV cache (block_size 16, 512 blocks).

Strategy:
  - Host side: scatter k/v into the paged cache via slot_mapping and gather
    per-sequence K/V via block_tables (pure permutation / shard preparation,
    exactly the reference semantics). Then shard one sequence per core and
    pre-lay-out the operands for the device kernel: Q^T [H*D, S] and
    K^T [KVH*D, S] in bf16 (so the PE needs no on-device transposes), and
    V augmented with a ones-column per kv-head, pre-swizzled to the exact
    [128, NT*(D+1)] SBUF layout (so every staging DMA is dense).
  - Device side (per core, SPMD): causal GQA attention for one sequence.
    scores^T [k, q] = K^T-block-stationary matmuls, bank-packed into
    [128, 1024] PSUM tiles. The exp is split across TWO engines so neither
    paces the kernel (ScalarE alone is ~162us busy = the old bottleneck):
      * tiles t0-t2 (2/3 of scores): ScalarE ACTIVATE Exp (exact)
      * tiles t3 (+t4 on even heads): DVE tensor_scalar Schraudolph
        exp: int16(round(s*(SCALE*log2e*128) + (16256-8))) bitcast to bf16
        is 2^(..) with ~1.5% rms mantissa error; verified rel-err 0.0055
        end to end on the exact problem inputs.
    Causal diag masking via gpsimd affine_select post-exp (bf16 view).
    P^T tiles are directly the PV stationary; V+ones moving operand gives
    the softmax denominator for free. Output normalized on DVE (batched
    reciprocal + broadcast multiply), written bf16 and unpacked on host.
  - Emission interleaves up to two PV output blocks after every QK score
    tile so the PE (the ~123us roofline engine: 296k stream cols @2.4GHz)
    never waits on score-buffer rotation; the end-of-kernel drain borrows
    the then-idle score-pool PSUM banks to deepen the psum_o rotation so
    the final PV blocks aren't paced by the DVE normalization latency.
"""

import numpy as np

B, S, H, KVH, D = 8, 1024, 32, 8, 128
G = H // KVH
NB, BS = 512, 16
T = B * S
SCALE = 0.08838834764831845
NCORES = 8
NT = S // 128
CB = 4

# Schraudolph constants: bf16 bits of exp(SCALE*s) ~= round(s*SC1 + SC2)
SC1 = SCALE * 1.4426950408889634 * 128.0
SC2 = 16256.0 - 8.0

_compiled = {}


def _build():
    import concourse.bass as bass
    import concourse.bacc as bacc
    import concourse.mybir as mybir
    import concourse.tile as tile

    f32 = mybir.dt.float32
    bf16 = mybir.dt.bfloat16
    i16 = mybir.dt.int16
    EXP = mybir.ActivationFunctionType.Exp

    nc = bacc.Bacc("TRN2", target_bir_lowering=False, debug=False,
                   num_devices=NCORES)
    qtd = nc.dram_tensor("qt", [H * D, S], bf16, kind="ExternalInput").ap()
    ktd = nc.dram_tensor("kt", [KVH * D, S], bf16, kind="ExternalInput").ap()
    vad = nc.dram_tensor("va", [128, KVH * NT * (D + 1)], bf16,
                         kind="ExternalInput").ap()
    # out slots: [(g*G+h4)*2 + c] -> [128, CB*D] (partition-major, dense)
    od = nc.dram_tensor("out", [128, H * 2, CB * D], bf16,
                        kind="ExternalOutput").ap()

    NCH = NT // CB

    # Bank-packed score-tile layouts covering BOTH q-chunks of one head:
    # every matmul stays inside a single 512-col (2KB) PSUM bank; offsets
    # pack the 512/384/256/128-wide causal blocks with no gaps.
    HPACK = [
        [(0, 0, 0), (0, 1, 512), (0, 3, 896)],                        # 1024
        [(1, 0, 0), (1, 1, 512)],                                     # 1024
        [(1, 2, 0), (1, 3, 512)],                                     # 1024
        [(1, 4, 0), (1, 5, 512), (1, 7, 896)],                        # 1024
        [(0, 2, 0), (1, 6, 256)],                                     # 512
    ]
    # Diagonal-mask specs per HPACK tile: (start col, block stride, nblocks).
    MASKS = {
        0: [(0, 512, 2), (896, 0, 1)],
        3: [(0, 512, 2), (896, 0, 1)],
        4: [(0, 256, 2)],
    }

    with tile.TileContext(nc) as tc:
        with (
            tc.tile_pool(name="ktp", bufs=KVH) as ktp,
            tc.tile_pool(name="qtp", bufs=H) as qtp,
            tc.tile_pool(name="vap", bufs=KVH) as vap,
            tc.tile_pool(name="pt", bufs=22) as ptp,
            tc.tile_pool(name="ost", bufs=8) as ostp,
            tc.tile_pool(name="small", bufs=6) as smallp,
            tc.tile_pool(name="psum_s", bufs=3, space="PSUM") as psum_s,
            tc.tile_pool(name="psum_o", bufs=2, space="PSUM") as psum_o,
        ):
            # ---- staging: all upfront; first group's first-needed slices
            # land first (and small) so the first QK matmul starts ASAP ----
            KTs, QTs, VAs = [], [], []
            for g in range(KVH):
                kt_t = ktp.tile([128, S], bf16, tag="kt")
                if g == 0:
                    nc.sync.dma_start(kt_t[:, 0:128], ktd[0:128, 0:128])
                else:
                    nc.sync.dma_start(kt_t[:], ktd[g * 128:(g + 1) * 128, :])
                qhs = []
                for h4 in range(G):
                    h = g * G + h4
                    qh_t = qtp.tile([128, S], bf16, tag="qt")
                    if g == 0 and h4 == 0:
                        nc.sync.dma_start(qh_t[:, 0:512], qtd[0:128, 0:512])
                        nc.sync.dma_start(kt_t[:, 128:S], ktd[0:128, 128:S])
                        nc.sync.dma_start(qh_t[:, 512:S], qtd[0:128, 512:S])
                    else:
                        nc.sync.dma_start(qh_t[:],
                                          qtd[h * 128:(h + 1) * 128, :])
                    qhs.append(qh_t)
                    if h4 == 0:
                        va_t = vap.tile([128, NT, D + 1], bf16, tag="va")
                        nc.sync.dma_start(
                            va_t[:],
                            vad[:, g * NT * (D + 1):(g + 1) * NT * (D + 1)]
                            .rearrange("p (n c) -> p n c", c=D + 1))
                KTs.append(kt_t)
                QTs.append(qhs)
                VAs.append(va_t)

            # warm path after the first staging issues: preload the Act
            # engine's Exp table (otherwise the ~2.7us ACT_TABLE_LOAD runs
            # serially before the first real ACTIVATE)
            fzero = nc.gpsimd.to_reg(0.0)
            warm = smallp.tile([128, 1], f32, tag="warm")
            nc.gpsimd.memset(warm[:], 0.0)
            nc.scalar.activation(warm[:], warm[:], EXP, scale=SCALE)

            def qk_tile(KT, QT, ti, pts, dve):
                # scores^T matmuls for one HPACK tile of one head; the
                # exp + mask emission is deferred (returned as a closure)
                # so per-slot DVE order is [norms of popped PV blocks]
                # before [exp] - freeing psum_o banks sooner.
                pack = HPACK[ti]
                st = psum_s.tile([128, 1024], f32, tag="st")
                w = 0
                for c, j, off in pack:
                    i0 = c * CB
                    n = (CB - max(j - i0, 0)) * 128
                    qcol = max(j, i0) * 128
                    nc.tensor.matmul(
                        st[:, off:off + n],
                        lhsT=KT[:, j * 128:(j + 1) * 128],
                        rhs=QT[:, qcol:qcol + n],
                        start=True, stop=True,
                    )
                    w = max(w, off + n)
                pt = ptp.tile([128, 1024], bf16, tag="pt")
                for c, j, off in pack:
                    pts[c, j] = (pt, off)

                def do_exp():
                    if dve:
                        nc.vector.tensor_scalar(
                            pt[:, :w].bitcast(i16), st[:, :w], SC1, SC2,
                            mybir.AluOpType.mult, mybir.AluOpType.add)
                    else:
                        nc.scalar.activation(pt[:, :w], st[:, :w], EXP,
                                             scale=SCALE)
                    for start, stride, nb in MASKS.get(ti, ()):
                        # zero strictly-lower (q < k) of the diag blocks
                        if nb == 1:
                            ap = pt[:, start:start + 128]
                            pat = [[1, 128]]
                        else:
                            ap = (pt[:, start:start + stride * nb]
                                  .rearrange("p (b x) -> p b x", x=stride)
                                  [:, :, 0:128])
                            pat = [[0, nb], [1, 128]]
                        nc.gpsimd.affine_select(
                            out=ap, in_=ap,
                            compare_op=mybir.AluOpType.is_ge,
                            fill=fzero, base=0, pattern=pat,
                            channel_multiplier=-1,
                        )
                return do_exp

            def pv_block(VA, pts, ost, c, h4, g, ii, hold, drain=False):
                # one 128-q output block of one (chunk, head): PSUM bank
                # accumulation; after the odd block of each half, batched
                # recip + normalize on DVE; after block 3, the out-DMA.
                # During the end-of-kernel drain the score pool is idle, so
                # borrow its banks to deepen the psum_o rotation.
                i0 = c * CB
                if ii % 2 == 0:
                    if drain:
                        st = psum_s.tile([128, 1024], f32, tag="st",
                                         name="st")
                        hold["o"] = st[:, 0:512]
                    else:
                        hold["o"] = psum_o.tile([128, 512], f32, tag="o",
                                                name="o")
                o = hold["o"]
                hi = ii % 2
                i = i0 + ii
                for j in range(i + 1):
                    pt, off = pts[c, j]
                    col = off + (ii - max(j - i0, 0)) * 128
                    nc.tensor.matmul(
                        o[:, hi * 256: hi * 256 + D + 1],
                        lhsT=pt[:, col:col + 128],
                        rhs=VA[:, j, :],
                        start=(j == 0), stop=(j == i),
                    )
                if hi == 1:
                    rec = smallp.tile([128, 2], f32, tag="rec")
                    nc.vector.reciprocal(rec[:], o[:, D::256])
                    ov = o[:].rearrange("p (b x) -> p b x", x=256)[:, :, 0:D]
                    rbc = (rec[:].rearrange("p b -> p b ()")
                           .broadcast_to((128, 2, D)))
                    nc.vector.tensor_tensor(
                        ost[:, ii - 1:ii + 1,
                            h4 * 128:(h4 + 1) * 128], ov, rbc,
                        mybir.AluOpType.mult)
                    if ii == 3:
                        slot = (g * G + h4) * 2 + c
                        nc.sync.dma_start(
                            od[:, slot, :],
                            ost[:, :, h4 * 128:(h4 + 1) * 128])

            # ---- main loop: per head, 5 QK score tiles; after each tile
            # emit one deferred PV half so the PE alternates QK/PV work and
            # never waits on score-buffer rotation ----
            pend = []
            osts = {}
            for g in range(KVH):
                ost1 = ostp.tile([128, CB, G * D], bf16, tag="ost")
                ost0 = ostp.tile([128, CB, G * D], bf16, tag="ost")
                osts = {1: ost1, 0: ost0}
                for h4 in range(G):
                    h = g * G + h4
                    pts = {}
                    for ti in range(5):
                        do_exp = qk_tile(
                            KTs[g], QTs[g][h4], ti, pts,
                            dve=(ti == 3) or (ti == 4 and h % 2 == 0))
                        do_exp()
                        npop = 0
                        while len(pend) >= 8 and npop < 2:
                            pv_block(*pend.pop(0))
                            npop += 1
                    for c in (1, 0):
                        for ii in range(4):
                            if ii % 2 == 0:
                                hold = {}
                            pend.append((VAs[g], pts, osts[c], c, h4, g,
                                         ii, hold))
            while pend:
                pv_block(*pend.pop(0), drain=len(pend) % 4 < 2)

    nc.compile()
    return nc


def _get_nc():
    if "nc" not in _compiled:
        _compiled["nc"] = _build()
    return _compiled["nc"]


def kernel(q, k, v, k_cache, v_cache, slot_mapping, block_tables):
    import ml_dtypes
    from concourse.bass_utils import run_bass_kernel_spmd

    bf16 = ml_dtypes.bfloat16
    q = np.ascontiguousarray(np.asarray(q, dtype=np.float32))
    k = np.asarray(k, dtype=np.float32)
    v = np.asarray(v, dtype=np.float32)
    sm = np.asarray(slot_mapping).astype(np.int64)
    bt = np.asarray(block_tables).astype(np.int64)

    # store_kvcache + page gather (reference semantics, pure permutation)
    kc = np.asarray(k_cache, dtype=np.float32).reshape(NB * BS, KVH * D).copy()
    vc = np.asarray(v_cache, dtype=np.float32).reshape(NB * BS, KVH * D).copy()
    kc[sm] = k
    vc[sm] = v
    kg = kc.reshape(NB, BS, KVH * D)[bt].reshape(B, S, KVH * D)
    vg = vc.reshape(NB, BS, KVH * D)[bt].reshape(B, S, KVH * D)

    # per-core device layouts: Q^T, K^T bf16; V+ones pre-swizzled to the
    # exact [128, KVH*NT*(D+1)] SBUF layout so staging DMAs are dense
    qT = q.reshape(B, S, H * D).transpose(0, 2, 1).astype(bf16)   # [B,H*D,S]
    kT = kg.transpose(0, 2, 1).astype(bf16)                       # [B,KVH*D,S]
    va = np.ones((B, S, KVH, D + 1), dtype=np.float32)
    va[:, :, :, :D] = vg.reshape(B, S, KVH, D)
    # [B, S=NT*128, KVH, D+1] -> [B, 128, KVH, NT, D+1]
    va = (va.reshape(B, NT, 128, KVH, D + 1)
          .transpose(0, 2, 3, 1, 4)
          .reshape(B, 128, KVH * NT * (D + 1)).astype(bf16))

    in_maps = [
        {"qt": np.ascontiguousarray(qT[i]),
         "kt": np.ascontiguousarray(kT[i]),
         "va": np.ascontiguousarray(va[i])}
        for i in range(NCORES)
    ]
    nc = _get_nc()
    res = run_bass_kernel_spmd(nc, in_maps, core_ids=list(range(NCORES)))
    _compiled["last_result"] = res
    # device out: [128, (g*G+h4)*2+c, CB*D]; q = c*512 + b*128 + p
    out = np.empty((NCORES, S, H * D), dtype=np.float32)
    for i in range(NCORES):
        o = np.asarray(res.results[i]["out"]).astype(np.float32)
        o = o.reshape(128, H, 2, CB, D)          # p, head, c, b, d
        o = o.transpose(2, 3, 0, 1, 4)           # c, b, p, head, d
        out[i] = o.reshape(S, H * D)
    return out.reshape(T, H * D)


# revision 16
# speedup vs baseline: 1.1758x; 1.1758x over previous
"""Paged causal GQA attention on 8 TRN2 NeuronCores.

Problem (hardcoded): B=8 seqs x S=1024 tokens, H=32 q-heads, KVH=8 kv-heads
(GQA group 4), D=128, f32 in/out, paged KV cache (block_size 16, 512 blocks).

Strategy:
  - Host side: scatter k/v into the paged cache via slot_mapping and gather
    per-sequence K/V via block_tables (pure permutation / shard preparation,
    exactly the reference semantics). Then shard one sequence per core and
    pre-lay-out the operands for the device kernel: Q^T [H*D, S] and
    K^T [KVH*D, S] in bf16 (so the PE needs no on-device transposes), and
    V augmented with a ones-column per kv-head, pre-swizzled to the exact
    [128, NT*(D+1)] SBUF layout (so every staging DMA is dense).
  - Device side (per core, SPMD): causal GQA attention for one sequence.
    scores^T [k, q] = K^T-block-stationary matmuls, bank-packed into
    [128, 1024] PSUM tiles. The exp is split across TWO engines so neither
    paces the kernel (ScalarE alone is ~162us busy = the old bottleneck):
      * tiles t0-t2 (2/3 of scores): ScalarE ACTIVATE Exp (exact)
      * tiles t3 (+t4 on even heads): DVE tensor_scalar Schraudolph
        exp: int16(round(s*(SCALE*log2e*128) + (16256-8))) bitcast to bf16
        is 2^(..) with ~1.5% rms mantissa error; verified rel-err 0.0055
        end to end on the exact problem inputs.
    Causal diag masking via gpsimd affine_select post-exp (bf16 view).
    P^T tiles are directly the PV stationary; V+ones moving operand gives
    the softmax denominator for free. Output normalized on DVE (batched
    reciprocal + broadcast multiply), written bf16 and unpacked on host.
  - Emission interleaves up to two PV output blocks after every QK score
    tile so the PE (the ~123us roofline engine: 296k stream cols @2.4GHz)
    never waits on score-buffer rotation; the end-of-kernel drain borrows
    the then-idle score-pool PSUM banks to deepen the psum_o rotation so
    the final PV blocks aren't paced by the DVE normalization latency.
"""

import numpy as np

B, S, H, KVH, D = 8, 1024, 32, 8, 128
G = H // KVH
NB, BS = 512, 16
T = B * S
SCALE = 0.08838834764831845
NCORES = 8
NT = S // 128
CB = 4

# Schraudolph constants: bf16 bits of exp(SCALE*s) ~= round(s*SC1 + SC2)
SC1 = SCALE * 1.4426950408889634 * 128.0
SC2 = 16256.0 - 8.0

_compiled = {}


def _build():
    import concourse.bass as bass
    import concourse.bacc as bacc
    import concourse.mybir as mybir
    import concourse.tile as tile

    f32 = mybir.dt.float32
    bf16 = mybir.dt.bfloat16
    i16 = mybir.dt.int16
    EXP = mybir.ActivationFunctionType.Exp

    nc = bacc.Bacc("TRN2", target_bir_lowering=False, debug=False,
                   num_devices=NCORES)
    qtd = nc.dram_tensor("qt", [H * D, S], bf16, kind="ExternalInput").ap()
    ktd = nc.dram_tensor("kt", [KVH * D, S], bf16, kind="ExternalInput").ap()
    vad = nc.dram_tensor("va", [128, KVH * NT * (D + 1)], bf16,
                         kind="ExternalInput").ap()
    # out slots: [(g*G+h4)*2 + c] -> [128, CB*D] (partition-major, dense)
    od = nc.dram_tensor("out", [128, H * 2, CB * D], bf16,
                        kind="ExternalOutput").ap()

    NCH = NT // CB

    # Bank-packed score-tile layouts covering BOTH q-chunks of one head:
    # every matmul stays inside a single 512-col (2KB) PSUM bank; offsets
    # pack the 512/384/256/128-wide causal blocks with no gaps.
    HPACK = [
        [(0, 0, 0), (0, 1, 512), (0, 3, 896)],                        # 1024
        [(1, 0, 0), (1, 1, 512)],                                     # 1024
        [(1, 2, 0), (1, 3, 512)],                                     # 1024
        [(1, 4, 0), (1, 5, 512), (1, 7, 896)],                        # 1024
        [(0, 2, 0), (1, 6, 256)],                                     # 512
    ]
    # Diagonal-mask specs per HPACK tile: (start col, block stride, nblocks).
    MASKS = {
        0: [(0, 512, 2), (896, 0, 1)],
        3: [(0, 512, 2), (896, 0, 1)],
        4: [(0, 256, 2)],
    }

    with tile.TileContext(nc) as tc:
        with (
            tc.tile_pool(name="ktp", bufs=KVH) as ktp,
            tc.tile_pool(name="qtp", bufs=H) as qtp,
            tc.tile_pool(name="vap", bufs=KVH) as vap,
            tc.tile_pool(name="pt", bufs=22) as ptp,
            tc.tile_pool(name="ost", bufs=8) as ostp,
            tc.tile_pool(name="small", bufs=6) as smallp,
            tc.tile_pool(name="psum_s", bufs=3, space="PSUM") as psum_s,
            tc.tile_pool(name="psum_o", bufs=2, space="PSUM") as psum_o,
        ):
            # ---- staging: all upfront; first group's first-needed slices
            # land first (and small) so the first QK matmul starts ASAP ----
            KTs, QTs, VAs = [], [], []
            for g in range(KVH):
                kt_t = ktp.tile([128, S], bf16, tag="kt")
                if g == 0:
                    nc.sync.dma_start(kt_t[:, 0:128], ktd[0:128, 0:128])
                else:
                    nc.sync.dma_start(kt_t[:], ktd[g * 128:(g + 1) * 128, :])
                qhs = []
                for h4 in range(G):
                    h = g * G + h4
                    qh_t = qtp.tile([128, S], bf16, tag="qt")
                    if g == 0 and h4 == 0:
                        nc.sync.dma_start(qh_t[:, 0:512], qtd[0:128, 0:512])
                        nc.sync.dma_start(kt_t[:, 128:S], ktd[0:128, 128:S])
                        nc.sync.dma_start(qh_t[:, 512:S], qtd[0:128, 512:S])
                    else:
                        nc.sync.dma_start(qh_t[:],
                                          qtd[h * 128:(h + 1) * 128, :])
                    qhs.append(qh_t)
                    if h4 == 0:
                        va_t = vap.tile([128, NT, D + 1], bf16, tag="va")
                        nc.sync.dma_start(
                            va_t[:],
                            vad[:, g * NT * (D + 1):(g + 1) * NT * (D + 1)]
                            .rearrange("p (n c) -> p n c", c=D + 1))
                KTs.append(kt_t)
                QTs.append(qhs)
                VAs.append(va_t)

            # warm path after the first staging issues: preload the Act
            # engine's Exp table (otherwise the ~2.7us ACT_TABLE_LOAD runs
            # serially before the first real ACTIVATE)
            fzero = nc.gpsimd.to_reg(0.0)
            warm = smallp.tile([128, 1], f32, tag="warm")
            nc.gpsimd.memset(warm[:], 0.0)
            nc.scalar.activation(warm[:], warm[:], EXP, scale=SCALE)

            def qk_tile(KT, QT, ti, pts, dve):
                # scores^T matmuls for one HPACK tile of one head; the
                # exp + mask emission is deferred (returned as a closure)
                # so per-slot DVE order is [norms of popped PV blocks]
                # before [exp] - freeing psum_o banks sooner.
                pack = HPACK[ti]
                st = psum_s.tile([128, 1024], f32, tag="st")
                w = 0
                for c, j, off in pack:
                    i0 = c * CB
                    n = (CB - max(j - i0, 0)) * 128
                    qcol = max(j, i0) * 128
                    nc.tensor.matmul(
                        st[:, off:off + n],
                        lhsT=KT[:, j * 128:(j + 1) * 128],
                        rhs=QT[:, qcol:qcol + n],
                        start=True, stop=True,
                    )
                    w = max(w, off + n)
                pt = ptp.tile([128, 1024], bf16, tag="pt")
                for c, j, off in pack:
                    pts[c, j] = (pt, off)

                def do_exp():
                    if dve:
                        nc.vector.tensor_scalar(
                            pt[:, :w].bitcast(i16), st[:, :w], SC1, SC2,
                            mybir.AluOpType.mult, mybir.AluOpType.add)
                    else:
                        nc.scalar.activation(pt[:, :w], st[:, :w], EXP,
                                             scale=SCALE)
                    for start, stride, nb in MASKS.get(ti, ()):
                        # zero strictly-lower (q < k) of the diag blocks
                        if nb == 1:
                            ap = pt[:, start:start + 128]
                            pat = [[1, 128]]
                        else:
                            ap = (pt[:, start:start + stride * nb]
                                  .rearrange("p (b x) -> p b x", x=stride)
                                  [:, :, 0:128])
                            pat = [[0, nb], [1, 128]]
                        nc.gpsimd.affine_select(
                            out=ap, in_=ap,
                            compare_op=mybir.AluOpType.is_ge,
                            fill=fzero, base=0, pattern=pat,
                            channel_multiplier=-1,
                        )
                return do_exp

            def pv_block(VA, pts, ost, c, h4, g, ii, hold, drain=False):
                # one 128-q output block of one (chunk, head): PSUM bank
                # accumulation; after the odd block of each half, batched
                # recip + normalize on DVE; after block 3, the out-DMA.
                # During the end-of-kernel drain the score pool is idle, so
                # borrow its banks to deepen the psum_o rotation.
                i0 = c * CB
                if ii % 2 == 0:
                    if drain:
                        st = psum_s.tile([128, 1024], f32, tag="st",
                                         name="st")
                        hold["o"] = st[:, 0:512]
                    else:
                        hold["o"] = psum_o.tile([128, 512], f32, tag="o",
                                                name="o")
                o = hold["o"]
                hi = ii % 2
                i = i0 + ii
                for j in range(i + 1):
                    pt, off = pts[c, j]
                    col = off + (ii - max(j - i0, 0)) * 128
                    nc.tensor.matmul(
                        o[:, hi * 256: hi * 256 + D + 1],
                        lhsT=pt[:, col:col + 128],
                        rhs=VA[:, j, :],
                        start=(j == 0), stop=(j == i),
                    )
                if hi == 1:
                    rec = smallp.tile([128, 2], f32, tag="rec")
                    nc.vector.reciprocal(rec[:], o[:, D::256])
                    ov = o[:].rearrange("p (b x) -> p b x", x=256)[:, :, 0:D]
                    rbc = (rec[:].rearrange("p b -> p b ()")
                           .broadcast_to((128, 2, D)))
                    nc.vector.tensor_tensor(
                        ost[:, ii - 1:ii + 1,
                            h4 * 128:(h4 + 1) * 128], ov, rbc,
                        mybir.AluOpType.mult)
                    if ii == 3:
                        slot = (g * G + h4) * 2 + c
                        nc.sync.dma_start(
                            od[:, slot, :],
                            ost[:, :, h4 * 128:(h4 + 1) * 128])

            # ---- main loop: per head, 5 QK score tiles; after each tile
            # emit one deferred PV half so the PE alternates QK/PV work and
            # never waits on score-buffer rotation ----
            pend = []
            osts = {}
            for g in range(KVH):
                ost1 = ostp.tile([128, CB, G * D], bf16, tag="ost")
                ost0 = ostp.tile([128, CB, G * D], bf16, tag="ost")
                osts = {1: ost1, 0: ost0}
                for h4 in range(G):
                    h = g * G + h4
                    pts = {}
                    for ti in range(5):
                        do_exp = qk_tile(
                            KTs[g], QTs[g][h4], ti, pts,
                            dve=ti in (3, 4))
                        do_exp()
                        npop = 0
                        while len(pend) >= 8 and npop < 2:
                            pv_block(*pend.pop(0))
                            npop += 1
                    for c in (1, 0):
                        for ii in range(4):
                            if ii % 2 == 0:
                                hold = {}
                            pend.append((VAs[g], pts, osts[c], c, h4, g,
                                         ii, hold))
            while pend:
                pv_block(*pend.pop(0), drain=len(pend) % 4 < 2)

    nc.compile()
    return nc


def _get_nc():
    if "nc" not in _compiled:
        _compiled["nc"] = _build()
    return _compiled["nc"]


def kernel(q, k, v, k_cache, v_cache, slot_mapping, block_tables):
    import ml_dtypes
    from concourse.bass_utils import run_bass_kernel_spmd

    bf16 = ml_dtypes.bfloat16
    q = np.ascontiguousarray(np.asarray(q, dtype=np.float32))
    k = np.asarray(k, dtype=np.float32)
    v = np.asarray(v, dtype=np.float32)
    sm = np.asarray(slot_mapping).astype(np.int64)
    bt = np.asarray(block_tables).astype(np.int64)

    # store_kvcache + page gather (reference semantics, pure permutation)
    kc = np.asarray(k_cache, dtype=np.float32).reshape(NB * BS, KVH * D).copy()
    vc = np.asarray(v_cache, dtype=np.float32).reshape(NB * BS, KVH * D).copy()
    kc[sm] = k
    vc[sm] = v
    kg = kc.reshape(NB, BS, KVH * D)[bt].reshape(B, S, KVH * D)
    vg = vc.reshape(NB, BS, KVH * D)[bt].reshape(B, S, KVH * D)

    # per-core device layouts: Q^T, K^T bf16; V+ones pre-swizzled to the
    # exact [128, KVH*NT*(D+1)] SBUF layout so staging DMAs are dense
    qT = q.reshape(B, S, H * D).transpose(0, 2, 1).astype(bf16)   # [B,H*D,S]
    kT = kg.transpose(0, 2, 1).astype(bf16)                       # [B,KVH*D,S]
    va = np.ones((B, S, KVH, D + 1), dtype=np.float32)
    va[:, :, :, :D] = vg.reshape(B, S, KVH, D)
    # [B, S=NT*128, KVH, D+1] -> [B, 128, KVH, NT, D+1]
    va = (va.reshape(B, NT, 128, KVH, D + 1)
          .transpose(0, 2, 3, 1, 4)
          .reshape(B, 128, KVH * NT * (D + 1)).astype(bf16))

    in_maps = [
        {"qt": np.ascontiguousarray(qT[i]),
         "kt": np.ascontiguousarray(kT[i]),
         "va": np.ascontiguousarray(va[i])}
        for i in range(NCORES)
    ]
    nc = _get_nc()
    res = run_bass_kernel_spmd(nc, in_maps, core_ids=list(range(NCORES)))
    _compiled["last_result"] = res
    # device out: [128, (g*G+h4)*2+c, CB*D]; q = c*512 + b*128 + p
    out = np.empty((NCORES, S, H * D), dtype=np.float32)
    for i in range(NCORES):
        o = np.asarray(res.results[i]["out"]).astype(np.float32)
        o = o.reshape(128, H, 2, CB, D)          # p, head, c, b, d
        o = o.transpose(2, 3, 0, 1, 4)           # c, b, p, head, d
        out[i] = o.reshape(S, H * D)
    return out.reshape(T, H * D)


# revision 17
# speedup vs baseline: 1.1968x; 1.0179x over previous
"""Paged causal GQA attention on 8 TRN2 NeuronCores.

Problem (hardcoded): B=8 seqs x S=1024 tokens, H=32 q-heads, KVH=8 kv-heads
(GQA group 4), D=128, f32 in/out, paged KV cache (block_size 16, 512 blocks).

Strategy:
  - Host side: scatter k/v into the paged cache via slot_mapping and gather
    per-sequence K/V via block_tables (pure permutation / shard preparation,
    exactly the reference semantics). Then shard one sequence per core and
    pre-lay-out the operands for the device kernel: Q^T [H*D, S] and
    K^T [KVH*D, S] in bf16 (so the PE needs no on-device transposes), and
    V augmented with a ones-column per kv-head, pre-swizzled to the exact
    [128, NT*(D+1)] SBUF layout (so every staging DMA is dense).
  - Device side (per core, SPMD): causal GQA attention for one sequence.
    scores^T [k, q] = K^T-block-stationary matmuls, bank-packed into
    [128, 1024] PSUM tiles. The exp is split across TWO engines so neither
    paces the kernel (ScalarE alone is ~162us busy = the old bottleneck):
      * tiles t0-t2 (2/3 of scores): ScalarE ACTIVATE Exp (exact)
      * tiles t3 (+t4 on even heads): DVE tensor_scalar Schraudolph
        exp: int16(round(s*(SCALE*log2e*128) + (16256-8))) bitcast to bf16
        is 2^(..) with ~1.5% rms mantissa error; verified rel-err 0.0055
        end to end on the exact problem inputs.
    Causal diag masking via gpsimd affine_select post-exp (bf16 view).
    P^T tiles are directly the PV stationary; V+ones moving operand gives
    the softmax denominator for free. Output normalized on DVE (batched
    reciprocal + broadcast multiply), written bf16 and unpacked on host.
  - Emission interleaves up to two PV output blocks after every QK score
    tile so the PE (the ~123us roofline engine: 296k stream cols @2.4GHz)
    never waits on score-buffer rotation; the end-of-kernel drain borrows
    the then-idle score-pool PSUM banks to deepen the psum_o rotation so
    the final PV blocks aren't paced by the DVE normalization latency.
"""

import numpy as np

B, S, H, KVH, D = 8, 1024, 32, 8, 128
G = H // KVH
NB, BS = 512, 16
T = B * S
SCALE = 0.08838834764831845
NCORES = 8
NT = S // 128
CB = 4

# Schraudolph constants: bf16 bits of exp(SCALE*s) ~= round(s*SC1 + SC2)
SC1 = SCALE * 1.4426950408889634 * 128.0
SC2 = 16256.0 - 8.0

_compiled = {}


def _build():
    import concourse.bass as bass
    import concourse.bacc as bacc
    import concourse.mybir as mybir
    import concourse.tile as tile

    f32 = mybir.dt.float32
    bf16 = mybir.dt.bfloat16
    i16 = mybir.dt.int16
    EXP = mybir.ActivationFunctionType.Exp

    nc = bacc.Bacc("TRN2", target_bir_lowering=False, debug=False,
                   num_devices=NCORES)
    qtd = nc.dram_tensor("qt", [H * D, S], bf16, kind="ExternalInput").ap()
    ktd = nc.dram_tensor("kt", [KVH * D, S], bf16, kind="ExternalInput").ap()
    vad = nc.dram_tensor("va", [128, KVH * NT * (D + 1)], bf16,
                         kind="ExternalInput").ap()
    # out slots: [(g*G+h4)*2 + c] -> [128, CB*D] (partition-major, dense)
    od = nc.dram_tensor("out", [128, H * 2, CB * D], bf16,
                        kind="ExternalOutput").ap()

    NCH = NT // CB

    # Bank-packed score-tile layouts covering BOTH q-chunks of one head:
    # every matmul stays inside a single 512-col (2KB) PSUM bank; offsets
    # pack the 512/384/256/128-wide causal blocks with no gaps.
    HPACK = [
        [(0, 0, 0), (0, 1, 512), (0, 3, 896)],                        # 1024
        [(1, 0, 0), (1, 1, 512)],                                     # 1024
        [(1, 2, 0), (1, 3, 512)],                                     # 1024
        [(1, 4, 0), (1, 5, 512), (1, 7, 896)],                        # 1024
        [(0, 2, 0), (1, 6, 256)],                                     # 512
    ]
    # Diagonal-mask specs per HPACK tile: (start col, block stride, nblocks).
    MASKS = {
        0: [(0, 512, 2), (896, 0, 1)],
        3: [(0, 512, 2), (896, 0, 1)],
        4: [(0, 256, 2)],
    }

    with tile.TileContext(nc) as tc:
        with (
            tc.tile_pool(name="ktp", bufs=KVH) as ktp,
            tc.tile_pool(name="qtp", bufs=H) as qtp,
            tc.tile_pool(name="vap", bufs=KVH) as vap,
            tc.tile_pool(name="pt", bufs=22) as ptp,
            tc.tile_pool(name="ost", bufs=8) as ostp,
            tc.tile_pool(name="small", bufs=6) as smallp,
            tc.tile_pool(name="psum_s", bufs=3, space="PSUM") as psum_s,
            tc.tile_pool(name="psum_o", bufs=2, space="PSUM") as psum_o,
        ):
            # ---- staging: all upfront; first group's first-needed slices
            # land first (and small) so the first QK matmul starts ASAP ----
            KTs, QTs, VAs = [], [], []
            for g in range(KVH):
                kt_t = ktp.tile([128, S], bf16, tag="kt")
                if g == 0:
                    nc.sync.dma_start(kt_t[:, 0:128], ktd[0:128, 0:128])
                else:
                    nc.sync.dma_start(kt_t[:], ktd[g * 128:(g + 1) * 128, :])
                qhs = []
                for h4 in range(G):
                    h = g * G + h4
                    qh_t = qtp.tile([128, S], bf16, tag="qt")
                    if g == 0 and h4 == 0:
                        nc.sync.dma_start(qh_t[:, 0:512], qtd[0:128, 0:512])
                        nc.sync.dma_start(kt_t[:, 128:S], ktd[0:128, 128:S])
                        nc.sync.dma_start(qh_t[:, 512:S], qtd[0:128, 512:S])
                    else:
                        nc.sync.dma_start(qh_t[:],
                                          qtd[h * 128:(h + 1) * 128, :])
                    qhs.append(qh_t)
                    if h4 == 0:
                        va_t = vap.tile([128, NT, D + 1], bf16, tag="va")
                        nc.sync.dma_start(
                            va_t[:],
                            vad[:, g * NT * (D + 1):(g + 1) * NT * (D + 1)]
                            .rearrange("p (n c) -> p n c", c=D + 1))
                KTs.append(kt_t)
                QTs.append(qhs)
                VAs.append(va_t)

            # warm path after the first staging issues: preload the Act
            # engine's Exp table (otherwise the ~2.7us ACT_TABLE_LOAD runs
            # serially before the first real ACTIVATE)
            fzero = nc.gpsimd.to_reg(0.0)
            warm = smallp.tile([128, 1], f32, tag="warm")
            nc.gpsimd.memset(warm[:], 0.0)
            nc.scalar.activation(warm[:], warm[:], EXP, scale=SCALE)

            def qk_tile(KT, QT, ti, pts, dve):
                # scores^T matmuls for one HPACK tile of one head; the
                # exp + mask emission is deferred (returned as a closure)
                # so per-slot DVE order is [norms of popped PV blocks]
                # before [exp] - freeing psum_o banks sooner.
                pack = HPACK[ti]
                st = psum_s.tile([128, 1024], f32, tag="st")
                w = 0
                for c, j, off in pack:
                    i0 = c * CB
                    n = (CB - max(j - i0, 0)) * 128
                    qcol = max(j, i0) * 128
                    nc.tensor.matmul(
                        st[:, off:off + n],
                        lhsT=KT[:, j * 128:(j + 1) * 128],
                        rhs=QT[:, qcol:qcol + n],
                        start=True, stop=True,
                    )
                    w = max(w, off + n)
                pt = ptp.tile([128, 1024], bf16, tag="pt")
                for c, j, off in pack:
                    pts[c, j] = (pt, off)

                def do_exp():
                    if dve:
                        nc.vector.tensor_scalar(
                            pt[:, :w].bitcast(i16), st[:, :w], SC1, SC2,
                            mybir.AluOpType.mult, mybir.AluOpType.add)
                    else:
                        nc.scalar.activation(pt[:, :w], st[:, :w], EXP,
                                             scale=SCALE)
                    for start, stride, nb in MASKS.get(ti, ()):
                        # zero strictly-lower (q < k) of the diag blocks
                        if nb == 1:
                            ap = pt[:, start:start + 128]
                            pat = [[1, 128]]
                        else:
                            ap = (pt[:, start:start + stride * nb]
                                  .rearrange("p (b x) -> p b x", x=stride)
                                  [:, :, 0:128])
                            pat = [[0, nb], [1, 128]]
                        nc.gpsimd.affine_select(
                            out=ap, in_=ap,
                            compare_op=mybir.AluOpType.is_ge,
                            fill=fzero, base=0, pattern=pat,
                            channel_multiplier=-1,
                        )
                return do_exp

            def pv_block(VA, pts, ost, c, h4, g, ii, hold, drain=False):
                # one 128-q output block of one (chunk, head): PSUM bank
                # accumulation; after the odd block of each half, batched
                # recip + normalize on DVE; after block 3, the out-DMA.
                # During the end-of-kernel drain the score pool is idle, so
                # borrow its banks to deepen the psum_o rotation.
                i0 = c * CB
                if ii % 2 == 0:
                    if drain:
                        st = psum_s.tile([128, 1024], f32, tag="st",
                                         name="st")
                        hold["o"] = st[:, 0:512]
                    else:
                        hold["o"] = psum_o.tile([128, 512], f32, tag="o",
                                                name="o")
                o = hold["o"]
                hi = ii % 2
                i = i0 + ii
                for j in range(i + 1):
                    pt, off = pts[c, j]
                    col = off + (ii - max(j - i0, 0)) * 128
                    nc.tensor.matmul(
                        o[:, hi * 256: hi * 256 + D + 1],
                        lhsT=pt[:, col:col + 128],
                        rhs=VA[:, j, :],
                        start=(j == 0), stop=(j == i),
                    )
                if hi == 1:
                    rec = smallp.tile([128, 2], f32, tag="rec")
                    nc.vector.reciprocal(rec[:], o[:, D::256])
                    ov = o[:].rearrange("p (b x) -> p b x", x=256)[:, :, 0:D]
                    rbc = (rec[:].rearrange("p b -> p b ()")
                           .broadcast_to((128, 2, D)))
                    nc.vector.tensor_tensor(
                        ost[:, ii - 1:ii + 1,
                            h4 * 128:(h4 + 1) * 128], ov, rbc,
                        mybir.AluOpType.mult)
                    if ii == 3:
                        slot = (g * G + h4) * 2 + c
                        nc.sync.dma_start(
                            od[:, slot, :],
                            ost[:, :, h4 * 128:(h4 + 1) * 128])

            # ---- main loop: per head, 5 QK score tiles; after each tile
            # emit one deferred PV half so the PE alternates QK/PV work and
            # never waits on score-buffer rotation ----
            pend = []
            osts = {}
            for g in range(KVH):
                ost1 = ostp.tile([128, CB, G * D], bf16, tag="ost")
                ost0 = ostp.tile([128, CB, G * D], bf16, tag="ost")
                osts = {1: ost1, 0: ost0}
                for h4 in range(G):
                    h = g * G + h4
                    pts = {}
                    for ti in range(5):
                        do_exp = qk_tile(
                            KTs[g], QTs[g][h4], ti, pts,
                            dve=(ti == 3) or (ti == 4 and h % 2 == 0))
                        do_exp()
                        npop = 0
                        while len(pend) >= 8 and npop < 2:
                            pv_block(*pend.pop(0))
                            npop += 1
                    for c in (1, 0):
                        for ii in range(4):
                            if ii % 2 == 0:
                                hold = {}
                            pend.append((VAs[g], pts, osts[c], c, h4, g,
                                         ii, hold))
            while pend:
                pv_block(*pend.pop(0), drain=len(pend) % 4 < 2)

    nc.compile()
    return nc


def _get_nc():
    if "nc" not in _compiled:
        _compiled["nc"] = _build()
    return _compiled["nc"]


def kernel(q, k, v, k_cache, v_cache, slot_mapping, block_tables):
    import ml_dtypes
    from concourse.bass_utils import run_bass_kernel_spmd

    bf16 = ml_dtypes.bfloat16
    q = np.ascontiguousarray(np.asarray(q, dtype=np.float32))
    k = np.asarray(k, dtype=np.float32)
    v = np.asarray(v, dtype=np.float32)
    sm = np.asarray(slot_mapping).astype(np.int64)
    bt = np.asarray(block_tables).astype(np.int64)

    # store_kvcache + page gather (reference semantics, pure permutation)
    kc = np.asarray(k_cache, dtype=np.float32).reshape(NB * BS, KVH * D).copy()
    vc = np.asarray(v_cache, dtype=np.float32).reshape(NB * BS, KVH * D).copy()
    kc[sm] = k
    vc[sm] = v
    kg = kc.reshape(NB, BS, KVH * D)[bt].reshape(B, S, KVH * D)
    vg = vc.reshape(NB, BS, KVH * D)[bt].reshape(B, S, KVH * D)

    # per-core device layouts: Q^T, K^T bf16; V+ones pre-swizzled to the
    # exact [128, KVH*NT*(D+1)] SBUF layout so staging DMAs are dense
    qT = q.reshape(B, S, H * D).transpose(0, 2, 1).astype(bf16)   # [B,H*D,S]
    kT = kg.transpose(0, 2, 1).astype(bf16)                       # [B,KVH*D,S]
    va = np.ones((B, S, KVH, D + 1), dtype=np.float32)
    va[:, :, :, :D] = vg.reshape(B, S, KVH, D)
    # [B, S=NT*128, KVH, D+1] -> [B, 128, KVH, NT, D+1]
    va = (va.reshape(B, NT, 128, KVH, D + 1)
          .transpose(0, 2, 3, 1, 4)
          .reshape(B, 128, KVH * NT * (D + 1)).astype(bf16))

    in_maps = [
        {"qt": np.ascontiguousarray(qT[i]),
         "kt": np.ascontiguousarray(kT[i]),
         "va": np.ascontiguousarray(va[i])}
        for i in range(NCORES)
    ]
    nc = _get_nc()
    res = run_bass_kernel_spmd(nc, in_maps, core_ids=list(range(NCORES)))
    _compiled["last_result"] = res
    # device out: [128, (g*G+h4)*2+c, CB*D]; q = c*512 + b*128 + p
    out = np.empty((NCORES, S, H * D), dtype=np.float32)
    for i in range(NCORES):
        o = np.asarray(res.results[i]["out"]).astype(np.float32)
        o = o.reshape(128, H, 2, CB, D)          # p, head, c, b, d
        o = o.transpose(2, 3, 0, 1, 4)           # c, b, p, head, d
        out[i] = o.reshape(S, H * D)
    return out.reshape(T, H * D)


# revision 18
# speedup vs baseline: 1.1971x; 1.0002x over previous
"""Paged causal GQA attention on 8 TRN2 NeuronCores.

Problem (hardcoded): B=8 seqs x S=1024 tokens, H=32 q-heads, KVH=8 kv-heads
(GQA group 4), D=128, f32 in/out, paged KV cache (block_size 16, 512 blocks).

Strategy:
  - Host side: scatter k/v into the paged cache via slot_mapping and gather
    per-sequence K/V via block_tables (pure permutation / shard preparation,
    exactly the reference semantics). Then shard one sequence per core and
    pre-lay-out the operands for the device kernel: Q^T [H*D, S] and
    K^T [KVH*D, S] in bf16 (so the PE needs no on-device transposes), and
    V augmented with a ones-column per kv-head, pre-swizzled to the exact
    [128, NT*(D+1)] SBUF layout (so every staging DMA is dense).
  - Device side (per core, SPMD): causal GQA attention for one sequence.
    scores^T [k, q] = K^T-block-stationary matmuls, bank-packed into
    [128, 1024] PSUM tiles. The exp is split across TWO engines so neither
    paces the kernel (ScalarE alone is ~162us busy = the old bottleneck):
      * tiles t0-t2 (2/3 of scores): ScalarE ACTIVATE Exp (exact)
      * tiles t3 (+t4 on even heads): DVE tensor_scalar Schraudolph
        exp: int16(round(s*(SCALE*log2e*128) + (16256-8))) bitcast to bf16
        is 2^(..) with ~1.5% rms mantissa error; verified rel-err 0.0055
        end to end on the exact problem inputs.
    Causal diag masking via gpsimd affine_select post-exp (bf16 view).
    P^T tiles are directly the PV stationary; V+ones moving operand gives
    the softmax denominator for free. Output normalized on DVE (batched
    reciprocal + broadcast multiply), written bf16 and unpacked on host.
  - Emission interleaves up to two PV output blocks after every QK score
    tile so the PE (the ~123us roofline engine: 296k stream cols @2.4GHz)
    never waits on score-buffer rotation; the end-of-kernel drain borrows
    the then-idle score-pool PSUM banks to deepen the psum_o rotation so
    the final PV blocks aren't paced by the DVE normalization latency.
"""

import numpy as np

B, S, H, KVH, D = 8, 1024, 32, 8, 128
G = H // KVH
NB, BS = 512, 16
T = B * S
SCALE = 0.08838834764831845
NCORES = 8
NT = S // 128
CB = 4

# Schraudolph constants: bf16 bits of exp(SCALE*s) ~= round(s*SC1 + SC2)
SC1 = SCALE * 1.4426950408889634 * 128.0
SC2 = 16256.0 - 8.0

_compiled = {}


def _build():
    import concourse.bass as bass
    import concourse.bacc as bacc
    import concourse.mybir as mybir
    import concourse.tile as tile

    f32 = mybir.dt.float32
    bf16 = mybir.dt.bfloat16
    i16 = mybir.dt.int16
    EXP = mybir.ActivationFunctionType.Exp

    nc = bacc.Bacc("TRN2", target_bir_lowering=False, debug=False,
                   num_devices=NCORES)
    qtd = nc.dram_tensor("qt", [H * D, S], bf16, kind="ExternalInput").ap()
    ktd = nc.dram_tensor("kt", [KVH * D, S], bf16, kind="ExternalInput").ap()
    vad = nc.dram_tensor("va", [128, KVH * NT * (D + 1)], bf16,
                         kind="ExternalInput").ap()
    # out slots: [(g*G+h4)*2 + c] -> [128, CB*D] (partition-major, dense)
    od = nc.dram_tensor("out", [128, H * 2, CB * D], bf16,
                        kind="ExternalOutput").ap()

    NCH = NT // CB

    # Bank-packed score-tile layouts covering BOTH q-chunks of one head:
    # every matmul stays inside a single 512-col (2KB) PSUM bank; offsets
    # pack the 512/384/256/128-wide causal blocks with no gaps.
    HPACK = [
        [(0, 0, 0), (0, 1, 512), (0, 3, 896)],                        # 1024
        [(1, 0, 0), (1, 1, 512)],                                     # 1024
        [(1, 2, 0), (1, 3, 512)],                                     # 1024
        [(1, 4, 0), (1, 5, 512), (1, 7, 896)],                        # 1024
        [(0, 2, 0), (1, 6, 256)],                                     # 512
    ]
    # Diagonal-mask specs per HPACK tile: (start col, block stride, nblocks).
    MASKS = {
        0: [(0, 512, 2), (896, 0, 1)],
        3: [(0, 512, 2), (896, 0, 1)],
        4: [(0, 256, 2)],
    }

    with tile.TileContext(nc) as tc:
        with (
            tc.tile_pool(name="ktp", bufs=KVH) as ktp,
            tc.tile_pool(name="qtp", bufs=H) as qtp,
            tc.tile_pool(name="vap", bufs=KVH) as vap,
            tc.tile_pool(name="pt", bufs=22) as ptp,
            tc.tile_pool(name="ost", bufs=8) as ostp,
            tc.tile_pool(name="small", bufs=6) as smallp,
            tc.tile_pool(name="psum_s", bufs=3, space="PSUM") as psum_s,
            tc.tile_pool(name="psum_o", bufs=2, space="PSUM") as psum_o,
        ):
            # ---- staging: all upfront; first group's first-needed slices
            # land first (and small) so the first QK matmul starts ASAP ----
            KTs, QTs, VAs = [], [], []
            for g in range(KVH):
                kt_t = ktp.tile([128, S], bf16, tag="kt")
                if g == 0:
                    nc.sync.dma_start(kt_t[:, 0:128], ktd[0:128, 0:128])
                else:
                    nc.sync.dma_start(kt_t[:], ktd[g * 128:(g + 1) * 128, :])
                qhs = []
                for h4 in range(G):
                    h = g * G + h4
                    qh_t = qtp.tile([128, S], bf16, tag="qt")
                    if g == 0 and h4 == 0:
                        nc.sync.dma_start(qh_t[:, 0:512], qtd[0:128, 0:512])
                        nc.sync.dma_start(kt_t[:, 128:512],
                                          ktd[0:128, 128:512])
                        nc.sync.dma_start(qh_t[:, 512:S], qtd[0:128, 512:S])
                        nc.sync.dma_start(kt_t[:, 512:S], ktd[0:128, 512:S])
                    else:
                        nc.sync.dma_start(qh_t[:],
                                          qtd[h * 128:(h + 1) * 128, :])
                    qhs.append(qh_t)
                    if h4 == 0:
                        va_t = vap.tile([128, NT, D + 1], bf16, tag="va")
                        nc.sync.dma_start(
                            va_t[:],
                            vad[:, g * NT * (D + 1):(g + 1) * NT * (D + 1)]
                            .rearrange("p (n c) -> p n c", c=D + 1))
                KTs.append(kt_t)
                QTs.append(qhs)
                VAs.append(va_t)

            # warm path after the first staging issues: preload the Act
            # engine's Exp table (otherwise the ~2.7us ACT_TABLE_LOAD runs
            # serially before the first real ACTIVATE)
            fzero = nc.gpsimd.to_reg(0.0)
            warm = smallp.tile([128, 1], f32, tag="warm")
            nc.gpsimd.memset(warm[:], 0.0)
            nc.scalar.activation(warm[:], warm[:], EXP, scale=SCALE)

            def qk_tile(KT, QT, ti, pts, dve):
                # scores^T matmuls for one HPACK tile of one head; the
                # exp + mask emission is deferred (returned as a closure)
                # so per-slot DVE order is [norms of popped PV blocks]
                # before [exp] - freeing psum_o banks sooner.
                pack = HPACK[ti]
                st = psum_s.tile([128, 1024], f32, tag="st")
                w = 0
                for c, j, off in pack:
                    i0 = c * CB
                    n = (CB - max(j - i0, 0)) * 128
                    qcol = max(j, i0) * 128
                    nc.tensor.matmul(
                        st[:, off:off + n],
                        lhsT=KT[:, j * 128:(j + 1) * 128],
                        rhs=QT[:, qcol:qcol + n],
                        start=True, stop=True,
                    )
                    w = max(w, off + n)
                pt = ptp.tile([128, 1024], bf16, tag="pt")
                for c, j, off in pack:
                    pts[c, j] = (pt, off)

                def do_exp():
                    if dve:
                        nc.vector.tensor_scalar(
                            pt[:, :w].bitcast(i16), st[:, :w], SC1, SC2,
                            mybir.AluOpType.mult, mybir.AluOpType.add)
                    else:
                        nc.scalar.activation(pt[:, :w], st[:, :w], EXP,
                                             scale=SCALE)
                    for start, stride, nb in MASKS.get(ti, ()):
                        # zero strictly-lower (q < k) of the diag blocks
                        if nb == 1:
                            ap = pt[:, start:start + 128]
                            pat = [[1, 128]]
                        else:
                            ap = (pt[:, start:start + stride * nb]
                                  .rearrange("p (b x) -> p b x", x=stride)
                                  [:, :, 0:128])
                            pat = [[0, nb], [1, 128]]
                        nc.gpsimd.affine_select(
                            out=ap, in_=ap,
                            compare_op=mybir.AluOpType.is_ge,
                            fill=fzero, base=0, pattern=pat,
                            channel_multiplier=-1,
                        )
                return do_exp

            def pv_block(VA, pts, ost, c, h4, g, ii, hold, drain=False):
                # one 128-q output block of one (chunk, head): PSUM bank
                # accumulation; after the odd block of each half, batched
                # recip + normalize on DVE; after block 3, the out-DMA.
                # During the end-of-kernel drain the score pool is idle, so
                # borrow its banks to deepen the psum_o rotation.
                i0 = c * CB
                if ii % 2 == 0:
                    if drain:
                        st = psum_s.tile([128, 1024], f32, tag="st",
                                         name="st")
                        hold["o"] = st[:, 0:512]
                    else:
                        hold["o"] = psum_o.tile([128, 512], f32, tag="o",
                                                name="o")
                o = hold["o"]
                hi = ii % 2
                i = i0 + ii
                for j in range(i + 1):
                    pt, off = pts[c, j]
                    col = off + (ii - max(j - i0, 0)) * 128
                    nc.tensor.matmul(
                        o[:, hi * 256: hi * 256 + D + 1],
                        lhsT=pt[:, col:col + 128],
                        rhs=VA[:, j, :],
                        start=(j == 0), stop=(j == i),
                    )
                if hi == 1:
                    rec = smallp.tile([128, 2], f32, tag="rec")
                    nc.vector.reciprocal(rec[:], o[:, D::256])
                    ov = o[:].rearrange("p (b x) -> p b x", x=256)[:, :, 0:D]
                    rbc = (rec[:].rearrange("p b -> p b ()")
                           .broadcast_to((128, 2, D)))
                    nc.vector.tensor_tensor(
                        ost[:, ii - 1:ii + 1,
                            h4 * 128:(h4 + 1) * 128], ov, rbc,
                        mybir.AluOpType.mult)
                    if ii == 3:
                        slot = (g * G + h4) * 2 + c
                        nc.sync.dma_start(
                            od[:, slot, :],
                            ost[:, :, h4 * 128:(h4 + 1) * 128])

            # ---- main loop: per head, 5 QK score tiles; after each tile
            # emit one deferred PV half so the PE alternates QK/PV work and
            # never waits on score-buffer rotation ----
            pend = []
            osts = {}
            for g in range(KVH):
                ost1 = ostp.tile([128, CB, G * D], bf16, tag="ost")
                ost0 = ostp.tile([128, CB, G * D], bf16, tag="ost")
                osts = {1: ost1, 0: ost0}
                for h4 in range(G):
                    h = g * G + h4
                    pts = {}
                    for ti in range(5):
                        do_exp = qk_tile(
                            KTs[g], QTs[g][h4], ti, pts,
                            dve=(ti == 3) or (ti == 4 and h % 2 == 0))
                        do_exp()
                        npop = 0
                        while len(pend) >= 8 and npop < 2:
                            pv_block(*pend.pop(0))
                            npop += 1
                    for c in (1, 0):
                        for ii in range(4):
                            if ii % 2 == 0:
                                hold = {}
                            pend.append((VAs[g], pts, osts[c], c, h4, g,
                                         ii, hold))
            while pend:
                pv_block(*pend.pop(0), drain=len(pend) % 4 < 2)

    nc.compile()
    return nc


def _get_nc():
    if "nc" not in _compiled:
        _compiled["nc"] = _build()
    return _compiled["nc"]


def kernel(q, k, v, k_cache, v_cache, slot_mapping, block_tables):
    import ml_dtypes
    from concourse.bass_utils import run_bass_kernel_spmd

    bf16 = ml_dtypes.bfloat16
    q = np.ascontiguousarray(np.asarray(q, dtype=np.float32))
    k = np.asarray(k, dtype=np.float32)
    v = np.asarray(v, dtype=np.float32)
    sm = np.asarray(slot_mapping).astype(np.int64)
    bt = np.asarray(block_tables).astype(np.int64)

    # store_kvcache + page gather (reference semantics, pure permutation)
    kc = np.asarray(k_cache, dtype=np.float32).reshape(NB * BS, KVH * D).copy()
    vc = np.asarray(v_cache, dtype=np.float32).reshape(NB * BS, KVH * D).copy()
    kc[sm] = k
    vc[sm] = v
    kg = kc.reshape(NB, BS, KVH * D)[bt].reshape(B, S, KVH * D)
    vg = vc.reshape(NB, BS, KVH * D)[bt].reshape(B, S, KVH * D)

    # per-core device layouts: Q^T, K^T bf16; V+ones pre-swizzled to the
    # exact [128, KVH*NT*(D+1)] SBUF layout so staging DMAs are dense
    qT = q.reshape(B, S, H * D).transpose(0, 2, 1).astype(bf16)   # [B,H*D,S]
    kT = kg.transpose(0, 2, 1).astype(bf16)                       # [B,KVH*D,S]
    va = np.ones((B, S, KVH, D + 1), dtype=np.float32)
    va[:, :, :, :D] = vg.reshape(B, S, KVH, D)
    # [B, S=NT*128, KVH, D+1] -> [B, 128, KVH, NT, D+1]
    va = (va.reshape(B, NT, 128, KVH, D + 1)
          .transpose(0, 2, 3, 1, 4)
          .reshape(B, 128, KVH * NT * (D + 1)).astype(bf16))

    in_maps = [
        {"qt": np.ascontiguousarray(qT[i]),
         "kt": np.ascontiguousarray(kT[i]),
         "va": np.ascontiguousarray(va[i])}
        for i in range(NCORES)
    ]
    nc = _get_nc()
    res = run_bass_kernel_spmd(nc, in_maps, core_ids=list(range(NCORES)))
    _compiled["last_result"] = res
    # device out: [128, (g*G+h4)*2+c, CB*D]; q = c*512 + b*128 + p
    out = np.empty((NCORES, S, H * D), dtype=np.float32)
    for i in range(NCORES):
        o = np.asarray(res.results[i]["out"]).astype(np.float32)
        o = o.reshape(128, H, 2, CB, D)          # p, head, c, b, d
        o = o.transpose(2, 3, 0, 1, 4)           # c, b, p, head, d
        out[i] = o.reshape(S, H * D)
    return out.reshape(T, H * D)


# revision 19
# speedup vs baseline: 1.2079x; 1.0090x over previous
"""Paged causal GQA attention on 8 TRN2 NeuronCores.

Problem (hardcoded): B=8 seqs x S=1024 tokens, H=32 q-heads, KVH=8 kv-heads
(GQA group 4), D=128, f32 in/out, paged KV cache (block_size 16, 512 blocks).

Strategy:
  - Host side: scatter k/v into the paged cache via slot_mapping and gather
    per-sequence K/V via block_tables (pure permutation / shard preparation,
    exactly the reference semantics). Then shard one sequence per core and
    pre-lay-out the operands for the device kernel: Q^T [H*D, S] and
    K^T [KVH*D, S] in bf16 (so the PE needs no on-device transposes), and
    V augmented with a ones-column per kv-head, pre-swizzled to the exact
    [128, NT*(D+1)] SBUF layout (so every staging DMA is dense).
  - Device side (per core, SPMD): causal GQA attention for one sequence.
    scores^T [k, q] = K^T-block-stationary matmuls, bank-packed into
    [128, 1024] PSUM tiles. The exp is split across TWO engines so neither
    paces the kernel (ScalarE alone is ~162us busy = the old bottleneck):
      * tiles t0-t2 (2/3 of scores): ScalarE ACTIVATE Exp (exact)
      * tiles t3 (+t4 on even heads): DVE tensor_scalar Schraudolph
        exp: int16(round(s*(SCALE*log2e*128) + (16256-8))) bitcast to bf16
        is 2^(..) with ~1.5% rms mantissa error; verified rel-err 0.0055
        end to end on the exact problem inputs.
    Causal diag masking via gpsimd affine_select post-exp (bf16 view).
    P^T tiles are directly the PV stationary; V+ones moving operand gives
    the softmax denominator for free. Output normalized on DVE (batched
    reciprocal + broadcast multiply), written bf16 and unpacked on host.
  - Emission interleaves up to two PV output blocks after every QK score
    tile so the PE (the ~123us roofline engine: 296k stream cols @2.4GHz)
    never waits on score-buffer rotation; the end-of-kernel drain borrows
    the then-idle score-pool PSUM banks to deepen the psum_o rotation so
    the final PV blocks aren't paced by the DVE normalization latency.
"""

import numpy as np

B, S, H, KVH, D = 8, 1024, 32, 8, 128
G = H // KVH
NB, BS = 512, 16
T = B * S
SCALE = 0.08838834764831845
NCORES = 8
NT = S // 128
CB = 4

# Schraudolph constants: bf16 bits of exp(SCALE*s) ~= round(s*SC1 + SC2)
SC1 = SCALE * 1.4426950408889634 * 128.0
SC2 = 16256.0 - 8.0

_compiled = {}


def _build():
    import concourse.bass as bass
    import concourse.bacc as bacc
    import concourse.mybir as mybir
    import concourse.tile as tile

    f32 = mybir.dt.float32
    bf16 = mybir.dt.bfloat16
    i16 = mybir.dt.int16
    EXP = mybir.ActivationFunctionType.Exp

    nc = bacc.Bacc("TRN2", target_bir_lowering=False, debug=False,
                   num_devices=NCORES)
    qtd = nc.dram_tensor("qt", [H * D, S], bf16, kind="ExternalInput").ap()
    ktd = nc.dram_tensor("kt", [KVH * D, S], bf16, kind="ExternalInput").ap()
    vad = nc.dram_tensor("va", [128, KVH * NT * (D + 1)], bf16,
                         kind="ExternalInput").ap()
    # out slots: [(g*G+h4)*2 + c] -> [128, CB*D] (partition-major, dense)
    od = nc.dram_tensor("out", [128, H * 2, CB * D], bf16,
                        kind="ExternalOutput").ap()

    NCH = NT // CB

    # Bank-packed score-tile layouts covering BOTH q-chunks of one head:
    # every matmul stays inside a single 512-col (2KB) PSUM bank; offsets
    # pack the 512/384/256/128-wide causal blocks with no gaps.
    HPACK = [
        [(0, 0, 0), (0, 1, 512), (0, 3, 896)],                        # 1024
        [(1, 0, 0), (1, 1, 512)],                                     # 1024
        [(1, 2, 0), (1, 3, 512)],                                     # 1024
        [(1, 4, 0), (1, 5, 512), (1, 7, 896)],                        # 1024
        [(0, 2, 0), (1, 6, 256)],                                     # 512
    ]
    # Diagonal-mask specs per HPACK tile: (start col, block stride, nblocks).
    MASKS = {
        0: [(0, 512, 2), (896, 0, 1)],
        3: [(0, 512, 2), (896, 0, 1)],
        4: [(0, 256, 2)],
    }

    with tile.TileContext(nc) as tc:
        with (
            tc.tile_pool(name="ktp", bufs=KVH) as ktp,
            tc.tile_pool(name="qtp", bufs=H) as qtp,
            tc.tile_pool(name="vap", bufs=KVH) as vap,
            tc.tile_pool(name="pt", bufs=22) as ptp,
            tc.tile_pool(name="ost", bufs=8) as ostp,
            tc.tile_pool(name="small", bufs=6) as smallp,
            tc.tile_pool(name="psum_s", bufs=3, space="PSUM") as psum_s,
            tc.tile_pool(name="psum_o", bufs=2, space="PSUM") as psum_o,
        ):
            # ---- staging: all upfront; first group's first-needed slices
            # land first (and small) so the first QK matmul starts ASAP ----
            KTs, QTs, VAs = [], [], []
            for g in range(KVH):
                kt_t = ktp.tile([128, S], bf16, tag="kt")
                if g == 0:
                    nc.sync.dma_start(kt_t[:, 0:128], ktd[0:128, 0:128])
                else:
                    nc.sync.dma_start(kt_t[:], ktd[g * 128:(g + 1) * 128, :])
                qhs = []
                for h4 in range(G):
                    h = g * G + h4
                    qh_t = qtp.tile([128, S], bf16, tag="qt")
                    if g == 0 and h4 == 0:
                        nc.sync.dma_start(qh_t[:, 0:512], qtd[0:128, 0:512])
                        nc.sync.dma_start(kt_t[:, 128:512],
                                          ktd[0:128, 128:512])
                        nc.sync.dma_start(qh_t[:, 512:S], qtd[0:128, 512:S])
                        nc.sync.dma_start(kt_t[:, 512:S], ktd[0:128, 512:S])
                    else:
                        nc.sync.dma_start(qh_t[:],
                                          qtd[h * 128:(h + 1) * 128, :])
                    qhs.append(qh_t)
                    if h4 == 0:
                        va_t = vap.tile([128, NT, D + 1], bf16, tag="va")
                        nc.sync.dma_start(
                            va_t[:],
                            vad[:, g * NT * (D + 1):(g + 1) * NT * (D + 1)]
                            .rearrange("p (n c) -> p n c", c=D + 1))
                KTs.append(kt_t)
                QTs.append(qhs)
                VAs.append(va_t)

            # warm path after the first staging issues: preload the Act
            # engine's Exp table (otherwise the ~2.7us ACT_TABLE_LOAD runs
            # serially before the first real ACTIVATE)
            fzero = nc.gpsimd.to_reg(0.0)
            warm = smallp.tile([128, 1], f32, tag="warm")
            nc.gpsimd.memset(warm[:], 0.0)
            nc.scalar.activation(warm[:], warm[:], EXP, scale=SCALE)

            # PE warm-up: the HAM clock gate keeps the PE at 1.2 GHz until
            # ~3.4us of sustained activity. The first real matmul waits on
            # staging DMA until ~10.5us anyway, so burn dummy matmuls into
            # a scratch PSUM slot during the DMA head to enter the main
            # loop already at 2.4 GHz.
            wmm = smallp.tile([128, 128], bf16, tag="wmm")
            nc.gpsimd.memset(wmm[:], 0.0)
            wps = psum_s.tile([128, 1024], f32, tag="st", name="st")
            for _ in range(28):
                nc.tensor.matmul(wps[:, 0:128], lhsT=wmm[:], rhs=wmm[:],
                                 start=True, stop=True)

            def qk_tile(KT, QT, ti, pts, dve):
                # scores^T matmuls for one HPACK tile of one head; the
                # exp + mask emission is deferred (returned as a closure)
                # so per-slot DVE order is [norms of popped PV blocks]
                # before [exp] - freeing psum_o banks sooner.
                pack = HPACK[ti]
                st = psum_s.tile([128, 1024], f32, tag="st")
                w = 0
                for c, j, off in pack:
                    i0 = c * CB
                    n = (CB - max(j - i0, 0)) * 128
                    qcol = max(j, i0) * 128
                    nc.tensor.matmul(
                        st[:, off:off + n],
                        lhsT=KT[:, j * 128:(j + 1) * 128],
                        rhs=QT[:, qcol:qcol + n],
                        start=True, stop=True,
                    )
                    w = max(w, off + n)
                pt = ptp.tile([128, 1024], bf16, tag="pt")
                for c, j, off in pack:
                    pts[c, j] = (pt, off)

                def do_exp():
                    if dve:
                        nc.vector.tensor_scalar(
                            pt[:, :w].bitcast(i16), st[:, :w], SC1, SC2,
                            mybir.AluOpType.mult, mybir.AluOpType.add)
                    else:
                        nc.scalar.activation(pt[:, :w], st[:, :w], EXP,
                                             scale=SCALE)
                    for start, stride, nb in MASKS.get(ti, ()):
                        # zero strictly-lower (q < k) of the diag blocks
                        if nb == 1:
                            ap = pt[:, start:start + 128]
                            pat = [[1, 128]]
                        else:
                            ap = (pt[:, start:start + stride * nb]
                                  .rearrange("p (b x) -> p b x", x=stride)
                                  [:, :, 0:128])
                            pat = [[0, nb], [1, 128]]
                        nc.gpsimd.affine_select(
                            out=ap, in_=ap,
                            compare_op=mybir.AluOpType.is_ge,
                            fill=fzero, base=0, pattern=pat,
                            channel_multiplier=-1,
                        )
                return do_exp

            def pv_block(VA, pts, ost, c, h4, g, ii, hold, drain=False):
                # one 128-q output block of one (chunk, head): PSUM bank
                # accumulation; after the odd block of each half, batched
                # recip + normalize on DVE; after block 3, the out-DMA.
                # During the end-of-kernel drain the score pool is idle, so
                # borrow its banks to deepen the psum_o rotation.
                i0 = c * CB
                if ii % 2 == 0:
                    if drain:
                        st = psum_s.tile([128, 1024], f32, tag="st",
                                         name="st")
                        hold["o"] = st[:, 0:512]
                    else:
                        hold["o"] = psum_o.tile([128, 512], f32, tag="o",
                                                name="o")
                o = hold["o"]
                hi = ii % 2
                i = i0 + ii
                for j in range(i + 1):
                    pt, off = pts[c, j]
                    col = off + (ii - max(j - i0, 0)) * 128
                    nc.tensor.matmul(
                        o[:, hi * 256: hi * 256 + D + 1],
                        lhsT=pt[:, col:col + 128],
                        rhs=VA[:, j, :],
                        start=(j == 0), stop=(j == i),
                    )
                if hi == 1:
                    rec = smallp.tile([128, 2], f32, tag="rec")
                    nc.vector.reciprocal(rec[:], o[:, D::256])
                    ov = o[:].rearrange("p (b x) -> p b x", x=256)[:, :, 0:D]
                    rbc = (rec[:].rearrange("p b -> p b ()")
                           .broadcast_to((128, 2, D)))
                    nc.vector.tensor_tensor(
                        ost[:, ii - 1:ii + 1,
                            h4 * 128:(h4 + 1) * 128], ov, rbc,
                        mybir.AluOpType.mult)
                    if ii == 3:
                        slot = (g * G + h4) * 2 + c
                        nc.sync.dma_start(
                            od[:, slot, :],
                            ost[:, :, h4 * 128:(h4 + 1) * 128])

            # ---- main loop: per head, 5 QK score tiles; after each tile
            # emit one deferred PV half so the PE alternates QK/PV work and
            # never waits on score-buffer rotation ----
            pend = []
            osts = {}
            for g in range(KVH):
                ost1 = ostp.tile([128, CB, G * D], bf16, tag="ost")
                ost0 = ostp.tile([128, CB, G * D], bf16, tag="ost")
                osts = {1: ost1, 0: ost0}
                for h4 in range(G):
                    h = g * G + h4
                    pts = {}
                    for ti in range(5):
                        do_exp = qk_tile(
                            KTs[g], QTs[g][h4], ti, pts,
                            dve=(ti == 3) or (ti == 4 and h % 2 == 0))
                        do_exp()
                        npop = 0
                        while len(pend) >= 8 and npop < 2:
                            pv_block(*pend.pop(0))
                            npop += 1
                    for c in (1, 0):
                        for ii in range(4):
                            if ii % 2 == 0:
                                hold = {}
                            pend.append((VAs[g], pts, osts[c], c, h4, g,
                                         ii, hold))
            while pend:
                pv_block(*pend.pop(0), drain=len(pend) % 4 < 2)

    nc.compile()
    return nc


def _get_nc():
    if "nc" not in _compiled:
        _compiled["nc"] = _build()
    return _compiled["nc"]


def kernel(q, k, v, k_cache, v_cache, slot_mapping, block_tables):
    import ml_dtypes
    from concourse.bass_utils import run_bass_kernel_spmd

    bf16 = ml_dtypes.bfloat16
    q = np.ascontiguousarray(np.asarray(q, dtype=np.float32))
    k = np.asarray(k, dtype=np.float32)
    v = np.asarray(v, dtype=np.float32)
    sm = np.asarray(slot_mapping).astype(np.int64)
    bt = np.asarray(block_tables).astype(np.int64)

    # store_kvcache + page gather (reference semantics, pure permutation)
    kc = np.asarray(k_cache, dtype=np.float32).reshape(NB * BS, KVH * D).copy()
    vc = np.asarray(v_cache, dtype=np.float32).reshape(NB * BS, KVH * D).copy()
    kc[sm] = k
    vc[sm] = v
    kg = kc.reshape(NB, BS, KVH * D)[bt].reshape(B, S, KVH * D)
    vg = vc.reshape(NB, BS, KVH * D)[bt].reshape(B, S, KVH * D)

    # per-core device layouts: Q^T, K^T bf16; V+ones pre-swizzled to the
    # exact [128, KVH*NT*(D+1)] SBUF layout so staging DMAs are dense
    qT = q.reshape(B, S, H * D).transpose(0, 2, 1).astype(bf16)   # [B,H*D,S]
    kT = kg.transpose(0, 2, 1).astype(bf16)                       # [B,KVH*D,S]
    va = np.ones((B, S, KVH, D + 1), dtype=np.float32)
    va[:, :, :, :D] = vg.reshape(B, S, KVH, D)
    # [B, S=NT*128, KVH, D+1] -> [B, 128, KVH, NT, D+1]
    va = (va.reshape(B, NT, 128, KVH, D + 1)
          .transpose(0, 2, 3, 1, 4)
          .reshape(B, 128, KVH * NT * (D + 1)).astype(bf16))

    in_maps = [
        {"qt": np.ascontiguousarray(qT[i]),
         "kt": np.ascontiguousarray(kT[i]),
         "va": np.ascontiguousarray(va[i])}
        for i in range(NCORES)
    ]
    nc = _get_nc()
    res = run_bass_kernel_spmd(nc, in_maps, core_ids=list(range(NCORES)))
    _compiled["last_result"] = res
    # device out: [128, (g*G+h4)*2+c, CB*D]; q = c*512 + b*128 + p
    out = np.empty((NCORES, S, H * D), dtype=np.float32)
    for i in range(NCORES):
        o = np.asarray(res.results[i]["out"]).astype(np.float32)
        o = o.reshape(128, H, 2, CB, D)          # p, head, c, b, d
        o = o.transpose(2, 3, 0, 1, 4)           # c, b, p, head, d
        out[i] = o.reshape(S, H * D)
    return out.reshape(T, H * D)
